# revision 41
# baseline (speedup 1.0000x reference)
"""Paged GQA attention (sparse_attention nn_Attention_29867202576782) on 8 trn2 cores.

Data-parallel over the B=16 sequences (2 per core); 16 (seq, kv-head) pairs per
core, each attending 128 query-columns (SQ*G) over S_TOTAL=2048 keys.

The kernel is DMA-bound (the exclusive DMA-engine pool moves ~17MB of fp16
KV per core at ~360GB/s), so everything else is organized to keep that stream
saturated end-to-end and off every other engine's critical path:

- The host applies the KV-cache scatter while laying the cache out into
  per-(pair, chunk-range) slab units ([128, w] fp16, >=2KB contiguous per
  partition): each partition carries the unit's K^T block (chunk-interleaved
  s = p*16 + c so it matches V's partition order) followed by its V block.
- q ships pre-transposed and pre-scaled ([d, pair, q] fp16), so the PE does no
  transposes at all: per chunk one score matmul (stationary K^T chunk), one
  PV matmul (stationary P^T chunk) and a 1-column denominator matmul into a
  separate PSUM bank (sharing the o bank resets the o accumulation group on
  real hardware), all fp16 at 1 cycle/row.
- One exp activation per unit reads the whole score PSUM block (up to
  [128, 8*128], spanning 2 banks) to amortize the ACT fixed access latency.
- PV runs LAG units behind scores (software pipelining) so the PE never
  head-of-line blocks on the exp result and keeps its p-state streak.
- The last pair is split 8/6/2 chunks so the compute tail after the final
  slab transfer is short.
- Slab loads ride the SP queue (the first on the Pool queue, whose DGE lead
  is shorter); the single q load rides the Activation queue. Output stores
  ([128, 256] fp16 per pair-pair) are all issued AFTER the slab stream on the
  SP queue (one on Pool), so their transfers fill the DMA-idle window behind
  the last slab instead of delaying it in the DMA-engine FIFO - stores are
  latency-insensitive, slabs are not.

TimelineSim: 56.3us/core; DMA_ENGINES ~49.3us busy (the bottleneck, gapless
but for the final compute tail), ACT ~34us, PE ~29us busy.
"""

from contextlib import ExitStack

import numpy as np

import concourse.bass as bass
import concourse.mybir as mybir
import concourse.tile as tile
from concourse import bacc, bass_utils

# Problem dims (hardcoded per the harness contract)
B, SQ, S_TOTAL = 16, 32, 2048
H, HKV, D = 32, 8, 128
G = H // HKV
SCALE = 0.08838834764831845
N_CORES = 8
B_LOC = B // N_CORES               # 2 sequences per core

P = 128
NPAIR = B_LOC * HKV                # 16 (seq, kv-head) pairs per core
NC_TOT = S_TOTAL // P              # 16 s-chunks per pair, s = p*16 + c
LAG = 2                            # PV pipeline lag (in units)

# Unit schedule: (pair, first_chunk, n_chunks). Halves everywhere; the last
# pair is split 8/6/2 so the post-stream compute tail is short.
UNITS = []
for _i in range(NPAIR - 1):
    UNITS += [(_i, 0, 8), (_i, 8, 8)]
UNITS += [(NPAIR - 1, 0, 8), (NPAIR - 1, 8, 6), (NPAIR - 1, 14, 2)]


def _unit_width(ncc):
    return 2 * ncc * P               # K^T block + V block


W_TOTAL = sum(_unit_width(ncc) for _, _, ncc in UNITS)

F32 = mybir.dt.float32
F16 = mybir.dt.float16

_CACHED_NC = {}


def _build_nc():
    nc = bacc.Bacc("TRN2", target_bir_lowering=False, debug=False,
                   enable_asserts=False, num_devices=N_CORES)

    od = nc.dram_tensor("o", [NPAIR // 2, P, 2 * P], F16, kind="ExternalOutput").ap()
    qtd = nc.dram_tensor("qt", [P, NPAIR * P], F16, kind="ExternalInput").ap()
    kvd = nc.dram_tensor("kv", [P, W_TOTAL], F16, kind="ExternalInput").ap()

    with tile.TileContext(nc) as tc, ExitStack() as ctx:
        with (
            tc.tile_pool(name="singles", bufs=1) as singles,
            tc.tile_pool(name="kvp", bufs=8) as kv_pool,
            tc.tile_pool(name="pTp", bufs=6) as pT_pool,
            tc.tile_pool(name="oop", bufs=8) as oo_pool,
            tc.tile_pool(name="smp", bufs=4) as small_pool,
            tc.tile_pool(name="sps", bufs=2, space="PSUM") as s_pool,
            tc.tile_pool(name="ops", bufs=2, space="PSUM") as o_pool,
            tc.tile_pool(name="lps", bufs=2, space="PSUM") as l_pool,
        ):
            ones_col = singles.tile([P, 1], F16)
            nc.vector.memset(ones_col[:], 1.0)
            # q^T for all 16 pairs: [d, pair, q] - on the Activation queue so
            # the SP queue can start streaming KV slabs immediately.
            qt = singles.tile([P, NPAIR, P], F16)
            nc.scalar.dma_start(
                qt[:], qtd.rearrange("p (i j) -> p i j", i=NPAIR, j=P))

            done_oo = []
            NU = len(UNITS)
            offs = np.cumsum([0] + [_unit_width(ncc) for _, _, ncc in UNITS])
            pend = {}      # unit idx -> (kv_tile, pT_tile)
            live = {}      # per-pair / pair-pair live tiles
            for u in range(NU + LAG):
                if u < NU:
                    i, c0, ncc = UNITS[u]
                    w = _unit_width(ncc)
                    kv_t = kv_pool.tile([P, _unit_width(8)], F16, tag="kv")
                    dma_eng = nc.gpsimd if u == 0 else nc.sync
                    dma_eng.dma_start(kv_t[:, 0:w],
                                      kvd[:, int(offs[u]):int(offs[u]) + w])
                    kT = kv_t[:, 0:ncc * P].rearrange(
                        "p (c s) -> p c s", c=ncc, s=P)
                    sps = s_pool.tile([P, 8, P], F32, tag="sps")
                    for c in range(ncc):
                        nc.tensor.matmul(sps[:, c, :], kT[:, c, :],
                                         qt[:, i, :], start=True, stop=True)
                    pT = pT_pool.tile([P, 8, P], F16, tag="pT")
                    nc.scalar.activation(pT[:, 0:ncc, :], sps[:, 0:ncc, :],
                                         mybir.ActivationFunctionType.Exp)
                    pend[u] = (kv_t, pT)
                if u >= LAG:
                    v = u - LAG
                    i, c0, ncc = UNITS[v]
                    w = _unit_width(ncc)
                    kv_t, pT = pend.pop(v)
                    vv = kv_t[:, ncc * P:w].rearrange(
                        "p (c e) -> p c e", c=ncc, e=P)
                    if c0 == 0:
                        o_ps = o_pool.tile([P, P], F32, tag="ops")
                        l_ps = l_pool.tile([P, 1], F32, tag="lps")
                        live[("o", i)] = (o_ps, l_ps)
                    else:
                        o_ps, l_ps = live[("o", i)]
                    for c in range(ncc):
                        nc.tensor.matmul(o_ps[:], pT[:, c, :], vv[:, c, :],
                                         start=(c0 + c == 0),
                                         stop=(c0 + c == NC_TOT - 1))
                        nc.tensor.matmul(l_ps[:], pT[:, c, :], ones_col[:],
                                         start=(c0 + c == 0),
                                         stop=(c0 + c == NC_TOT - 1))
                    if c0 + ncc == NC_TOT:
                        del live[("o", i)]
                        linv = small_pool.tile([P, 1], F32, tag="linv")
                        nc.vector.reciprocal(linv[:], l_ps[:])
                        if i % 2 == 0:
                            oo = oo_pool.tile([P, 2 * P], F16, tag="oo")
                            live[("oo", i // 2)] = oo
                        else:
                            oo = live.pop(("oo", i // 2))
                        nc.vector.tensor_scalar_mul(
                            oo[:, (i % 2) * P:(i % 2 + 1) * P],
                            o_ps[:], linv[:])
                        if i % 2 == 1:
                            done_oo.append((i // 2, oo))

            for j, oo_p in done_oo:
                eng = nc.gpsimd if j == 6 else nc.sync
                eng.dma_start(od[j], oo_p[:])

    nc.compile()
    return nc


def get_nc():
    if "nc" not in _CACHED_NC:
        _CACHED_NC["nc"] = _build_nc()
    return _CACHED_NC["nc"]


def shard_inputs(q, k, v, k_cache, v_cache, slot_mapping):
    """Apply the KV scatter and build per-core slab/qT input maps."""
    k_new = np.asarray(k).reshape(-1, HKV, D)
    v_new = np.asarray(v).reshape(-1, HKV, D)
    sm = np.asarray(slot_mapping)
    kc4 = np.asarray(k_cache).reshape(B, S_TOTAL, HKV, D)
    vc4 = np.asarray(v_cache).reshape(B, S_TOTAL, HKV, D)
    q2 = np.asarray(q)

    in_maps = []
    for ci in range(N_CORES):
        b0 = B_LOC * ci
        kc = kc4[b0:b0 + B_LOC].copy()
        vc = vc4[b0:b0 + B_LOC].copy()
        lo, hi = b0 * S_TOTAL, (b0 + B_LOC) * S_TOTAL
        msk = (sm >= lo) & (sm < hi)
        if msk.any():
            idx = sm[msk] - lo
            kc.reshape(-1, HKV, D)[idx] = k_new[msk]
            vc.reshape(-1, HKV, D)[idx] = v_new[msk]

        # [b, s, hh, d] -> [pair, p, c, d] with s = p*16 + c
        def chunked(a):
            return (a.transpose(0, 2, 1, 3)
                     .reshape(NPAIR, P, NC_TOT, D))
        kh = chunked(kc).astype(np.float16)
        vh = chunked(vc).astype(np.float16)
        kT = kh.transpose(0, 3, 2, 1)            # [pair, d, c, p]

        parts = []
        for i, c0, ncc in UNITS:
            parts.append(kT[i, :, c0:c0 + ncc, :].reshape(P, ncc * P))
            parts.append(vh[i, :, c0:c0 + ncc, :].reshape(P, ncc * P))
        kv = np.concatenate(parts, axis=1)

        # q^T: [b, q, hh, g, d] -> [d, pair, q*G+g], pre-scaled
        qc = (q2[b0 * SQ:(b0 + B_LOC) * SQ]
              .reshape(B_LOC, SQ, HKV, G, D)
              .transpose(0, 2, 1, 3, 4)
              .reshape(NPAIR, SQ * G, D)
              .transpose(2, 0, 1)) * SCALE

        in_maps.append({
            "qt": np.ascontiguousarray(
                qc.reshape(P, NPAIR * P).astype(np.float16)),
            "kv": np.ascontiguousarray(kv),
        })
    return in_maps


def _unshard(results):
    outs = []
    for ci in range(N_CORES):
        o_dev = np.asarray(results[ci]["o"], dtype=np.float32)
        o_pair = (o_dev.reshape(NPAIR // 2, P, 2, P)
                  .transpose(0, 2, 1, 3)
                  .reshape(B_LOC, HKV, SQ, G, D)
                  .transpose(0, 2, 1, 3, 4)
                  .reshape(B_LOC * SQ, H * D))
        outs.append(o_pair)
    return np.concatenate(outs, axis=0)


def kernel(q, k, v, k_cache, v_cache, slot_mapping, _trace=False):
    in_maps = shard_inputs(q, k, v, k_cache, v_cache, slot_mapping)
    nc = get_nc()
    res = bass_utils.run_bass_kernel_spmd(
        nc, in_maps, core_ids=list(range(N_CORES)), trace=_trace)
    out = _unshard(res.results)
    if _trace:
        kernel.last_results = res
    return out
